# revision 33
# baseline (speedup 1.0000x reference)
"""Trainium2 Bass kernel for a GNN message-passing NodeModel.

reference semantics:
    agg = scatter_mean(e, edge_index[1], N)        # [N, h]
    x   = concat([agg, v], -1)                     # [N, 2h]
    out = (relu(relu(x@W0+b0)@W1+b1))@W2+b2        # [N, h]

Sharding: nodes are partitioned contiguously across the 8 cores
(6250 nodes/core).  Edges are routed (host side) to the core that owns
their *receiver*, so the segment-sum and the MLP are fully local per
core and no collective is needed.

Per core, receivers are grouped into W-node blocks (W=64).  Each
block's edges are packed into fixed-size tiles of 128 edges.  On
device, per block j:
  oh[p, t, n] = (rloc[p, t] == iota[n])            (ONE DVE tensor_tensor,
                                                    broadcast APs)
  ps[h, jW+n] += e_tile[p, h]^T @ oh[:, t, :]      (TensorE, f32 PSUM,
                                                    per 128-edge tile t)
Per 512-node chunk (8 blocks), the scatter-sum PSUM is turned into the
scatter-mean and written to SBUF in one DVE op:
  agg[h, n] = ps[h, n] * crecip[h, n]              (crecip = 1/max(deg,1)
                                                    replicated rows, from host)
The MLP then runs feature-major over the same chunks:
  h1 = relu(W0a^T@agg + W0b^T@v^T + b0); h2 = relu(W1^T@h1 + b1);
  out = W2^T@h2 + b2.
"""

import functools

import numpy as np
import ml_dtypes

import concourse.bass as bass
import concourse.mybir as mybir
from concourse import bacc
from concourse.tile import TileContext
from concourse.bass_utils import run_bass_kernel_spmd

BF16 = ml_dtypes.bfloat16

N_NODES = 50000
N_EDGES = 800000
H = 64
N_CORES = 8
NPC = N_NODES // N_CORES          # 6250 nodes per core
W = 64                            # node-block (one-hot window) size
NBLK = -(-NPC // W)               # 98 blocks per core
NPAD = NBLK * W                   # 6272 padded nodes per core
MLP_CH = 512                      # MLP chunk (nodes per chunk)
BLK_GRP = 2                       # node blocks fetched per edge DMA


# --------------------------------------------------------------------------
# Host-side packing
# --------------------------------------------------------------------------

def pack_inputs(v, edge_index, e, W0, b0, W1, b1, W2, b2,
                n_cores=N_CORES, npc=NPC, nblk=NBLK, npad=NPAD):
    """Shard + lay out the full inputs for the 8 cores.

    Returns (T, in_maps): T is the number of 128-edge tiles per node
    block, in_maps the list of per-core input dicts.
    """
    h = W0.shape[1]
    n_nodes = n_cores * npc
    n_edges = e.shape[0]

    recv = np.asarray(edge_index[1], dtype=np.int64)
    order = np.argsort(recv, kind="stable")
    recv_s = recv[order]

    counts = np.bincount(recv, minlength=n_nodes)
    crecip_node = (1.0 / np.maximum(counts, 1)).astype(np.float32)

    core = recv_s // npc
    loc = recv_s - core * npc
    blk = loc // W
    rloc = loc - blk * W                         # in [0, W)
    key = core * nblk + blk
    cnt_blk = np.bincount(key, minlength=n_cores * nblk)
    T = max(1, -(-int(cnt_blk.max()) // 128))

    starts = np.zeros(n_cores * nblk, np.int64)
    np.cumsum(cnt_blk[:-1], out=starts[1:])
    pos = np.arange(n_edges) - starts[key]
    p = pos % 128
    t = pos // 128

    arr = np.zeros((n_cores, nblk, 128, T, h), dtype=BF16)
    arr[core, blk, p, t] = np.asarray(e, np.float32)[order].astype(BF16)
    # group BLK_GRP adjacent blocks into one DMA record:
    # [nblk/G, 128, G*T*h] with each partition's line contiguous
    g = BLK_GRP
    arr = arr.reshape(n_cores, nblk // g, g, 128, T * h)
    arr = np.ascontiguousarray(arr.transpose(0, 1, 3, 2, 4))
    arr = arr.reshape(n_cores, nblk // g, 128, g * T * h)

    # per-edge local receiver index, bf16, [core, p, blk*T]
    meta = np.zeros((n_cores, 128, nblk, T), dtype=BF16)
    meta[core, p, blk, t] = rloc.astype(np.float32).astype(BF16)
    meta = meta.reshape(n_cores, 128, nblk * T)

    # int16 scatter indices t*W + rloc for the gpsimd local_scatter path,
    # padded to even T2 with -1 ("negative indices are ignored")
    T2 = T + (T % 2)
    sidx = np.full((n_cores, 128, nblk, T2), -1, dtype=np.int16)
    sidx[core, p, blk, t] = (t * W + rloc).astype(np.int16)
    sidx = sidx.reshape(n_cores, 128, nblk * T2)

    # 1/deg replicated down feature rows, [core, h, npad] bf16
    crp = np.zeros((n_cores, npad), np.float32)
    crp[:, :npc] = crecip_node.reshape(n_cores, npc)
    crecip = np.ascontiguousarray(
        np.broadcast_to(crp[:, None, :], (n_cores, h, npad))).astype(BF16)

    vp = np.zeros((n_cores, npad, h), np.float32)
    vp[:, :npc] = np.asarray(v, np.float32).reshape(n_cores, npc, h)
    vt = np.ascontiguousarray(vp.transpose(0, 2, 1)).astype(BF16)  # [C, h, npad]

    W0 = np.asarray(W0, np.float32)
    w0a = np.ascontiguousarray(W0[:h]).astype(BF16)
    w0b = np.ascontiguousarray(W0[h:]).astype(BF16)
    w1 = np.asarray(W1, np.float32).astype(BF16)
    w2 = np.asarray(W2, np.float32).astype(BF16)
    b0 = np.asarray(b0, np.float32).reshape(h, 1).copy()
    b1 = np.asarray(b1, np.float32).reshape(h, 1).copy()
    b2 = np.asarray(b2, np.float32).reshape(h, 1).copy()

    in_maps = []
    for c in range(n_cores):
        in_maps.append({
            "arr": arr[c], "meta": meta[c], "sidx": sidx[c],
            "vt": vt[c], "crecip": crecip[c],
            "w0a": w0a, "w0b": w0b, "w1": w1, "w2": w2,
            "b0": b0, "b1": b1, "b2": b2,
        })
    return T, in_maps


# --------------------------------------------------------------------------
# Device kernel
# --------------------------------------------------------------------------

@functools.lru_cache(maxsize=4)
def build_kernel(T, nblk=NBLK, npad=NPAD):
    bf = mybir.dt.bfloat16
    f32 = mybir.dt.float32
    AF = mybir.ActivationFunctionType

    nc = bacc.Bacc(trn_type="TRN2", debug=False)

    g = BLK_GRP
    arr_d = nc.declare_dram_parameter("arr", [nblk // g, 128, g * T * H], bf,
                                      isOutput=False)
    meta_d = nc.declare_dram_parameter("meta", [128, nblk * T], bf, isOutput=False)
    T2 = T + (T % 2)
    sidx_d = nc.declare_dram_parameter("sidx", [128, nblk * T2], mybir.dt.int16,
                                       isOutput=False)
    vt_d = nc.declare_dram_parameter("vt", [H, npad], bf, isOutput=False)
    cr_d = nc.declare_dram_parameter("crecip", [H, npad], bf, isOutput=False)
    w0a_d = nc.declare_dram_parameter("w0a", [H, H], bf, isOutput=False)
    w0b_d = nc.declare_dram_parameter("w0b", [H, H], bf, isOutput=False)
    w1_d = nc.declare_dram_parameter("w1", [H, H], bf, isOutput=False)
    w2_d = nc.declare_dram_parameter("w2", [H, H], bf, isOutput=False)
    b0_d = nc.declare_dram_parameter("b0", [H, 1], f32, isOutput=False)
    b1_d = nc.declare_dram_parameter("b1", [H, 1], f32, isOutput=False)
    b2_d = nc.declare_dram_parameter("b2", [H, 1], f32, isOutput=False)
    out_d = nc.declare_dram_parameter("out", [H, npad], f32, isOutput=True)

    n_ch = -(-npad // MLP_CH)
    blocks_per_ch = MLP_CH // W          # 8

    with TileContext(nc) as tc:
        with (
            tc.tile_pool(name="const", bufs=1) as cpool,
            tc.tile_pool(name="blocks", bufs=4) as bpool,
            tc.tile_pool(name="oh", bufs=10) as ohpool,
            tc.tile_pool(name="mlp", bufs=3) as mpool,
            tc.tile_pool(name="pseg", bufs=2, space="PSUM") as psegp,
            tc.tile_pool(name="pmlp", bufs=2, space="PSUM") as pmlpp,
        ):
            # iota row [0..W-1] per partition, built on-device
            iota_i16 = cpool.tile([128, W], mybir.dt.int16)
            nc.gpsimd.iota(out=iota_i16[:], pattern=[[1, W]], base=0,
                           channel_multiplier=0)
            iota_sb = cpool.tile([128, W], bf)
            nc.vector.tensor_copy(out=iota_sb[:], in_=iota_i16[:])
            meta_sb = cpool.tile([128, nblk * T], bf)
            nc.sync.dma_start(out=meta_sb[:], in_=meta_d[:])
            # absorb the meta DMA wait on DVE before the eq ops
            meta_probe = cpool.tile([128, 1], bf)
            nc.vector.tensor_copy(out=meta_probe[:], in_=meta_sb[:, 0:1])
            sidx_sb = cpool.tile([128, nblk * T2], mybir.dt.int16)
            nc.sync.dma_start(out=sidx_sb[:], in_=sidx_d[:])
            ones_sb = cpool.tile([128, T2], bf)
            nc.gpsimd.memset(ones_sb[:], 1.0)
            from concourse import library_config
            nc.gpsimd.load_library(library_config.local_scatter)

            vt_sb = cpool.tile([H, npad], bf)
            nc.scalar.dma_start(out=vt_sb[:], in_=vt_d[:])
            cr_sb = cpool.tile([H, npad], bf)
            nc.scalar.dma_start(out=cr_sb[:], in_=cr_d[:])
            w0a_sb = cpool.tile([H, H], bf)
            nc.scalar.dma_start(out=w0a_sb[:], in_=w0a_d[:])
            w0b_sb = cpool.tile([H, H], bf)
            nc.scalar.dma_start(out=w0b_sb[:], in_=w0b_d[:])
            w1_sb = cpool.tile([H, H], bf)
            nc.scalar.dma_start(out=w1_sb[:], in_=w1_d[:])
            w2_sb = cpool.tile([H, H], bf)
            nc.scalar.dma_start(out=w2_sb[:], in_=w2_d[:])
            b0_sb = cpool.tile([H, 1], f32)
            nc.scalar.dma_start(out=b0_sb[:], in_=b0_d[:])
            b1_sb = cpool.tile([H, 1], f32)
            nc.scalar.dma_start(out=b1_sb[:], in_=b1_d[:])
            b2_sb = cpool.tile([H, 1], f32)
            nc.scalar.dma_start(out=b2_sb[:], in_=b2_d[:])

            agg_tiles = [cpool.tile([H, MLP_CH], bf, tag=f"agg{i}",
                                    name=f"agg{i}")
                         for i in range(n_ch)]
            out_sb = cpool.tile([H, npad], f32)

            # ---- scatter-sum (per chunk of 8 blocks) + mean + MLP ----
            blk_tiles = {}

            def emit_oh(j):
                oh = ohpool.tile([128, T * W], bf, tag="oh", name="oh")
                if j < 2 * blocks_per_ch or j % 2 == 0:
                    rloc_ap = meta_sb[:, j * T:(j + 1) * T]  # [128, T]
                    nc.vector.tensor_tensor(
                        out=oh[:].rearrange("p (t w) -> p t w", w=W),
                        in0=rloc_ap.to_broadcast([128, T, W]),
                        in1=iota_sb[:].rearrange("p (o w) -> p o w", o=1)
                            .to_broadcast([128, T, W]),
                        op=mybir.AluOpType.is_equal,
                    )
                else:
                    nc.gpsimd.local_scatter(
                        out_ap=oh[:],
                        data_ap=ones_sb[:],
                        idxs_ap=sidx_sb[:, j * T2:(j + 1) * T2],
                        channels=128,
                        num_elems=T * W,
                        num_idxs=T2,
                    )
                return oh

            # smallest chunk first: PE's first matmuls are gated behind
            # only 2 one-hot ops instead of a full chunk's worth
            for ci in [n_ch - 1] + list(range(n_ch - 1)):
                ps = psegp.tile([H, MLP_CH], f32, tag="ps", name="ps")
                for bj in range(blocks_per_ch):
                    j = ci * blocks_per_ch + bj        # global block
                    if j >= nblk:
                        break
                    if j % g == 0:
                        blkt = bpool.tile([128, g * T * H], bf, tag="blk",
                                          name="blkt")
                        nc.sync.dma_start(out=blkt[:], in_=arr_d[j // g])
                        for i in range(g):
                            blk_tiles[j + i] = (blkt, i)
                    blkt, sub = blk_tiles[j]
                    base = sub * T * H
                    oh = emit_oh(j)
                    for t in range(T):
                        nc.tensor.matmul(
                            out=ps[:, bj * W:(bj + 1) * W],
                            lhsT=blkt[:, base + t * H: base + (t + 1) * H],
                            rhs=oh[:, t * W:(t + 1) * W],
                            start=(t == 0),
                            stop=(t == T - 1),
                        )
                c0 = ci * MLP_CH
                csz = min(MLP_CH, npad - c0)
                nc.vector.tensor_tensor(
                    out=agg_tiles[ci][:, :csz],
                    in0=ps[:, :csz],
                    in1=cr_sb[:, c0:c0 + csz],
                    op=mybir.AluOpType.mult,
                )
                h1p = pmlpp.tile([H, MLP_CH], f32, tag="ps1", name="h1p")
                nc.tensor.matmul(out=h1p[:, :csz], lhsT=w0a_sb[:],
                                 rhs=agg_tiles[ci][:, :csz],
                                 start=True, stop=False)
                nc.tensor.matmul(out=h1p[:, :csz], lhsT=w0b_sb[:],
                                 rhs=vt_sb[:, c0:c0 + csz],
                                 start=False, stop=True)
                h1 = mpool.tile([H, MLP_CH], bf, tag="h1", name="h1")
                nc.scalar.activation(out=h1[:, :csz], in_=h1p[:, :csz],
                                     func=AF.Relu, bias=b0_sb[:])
                h2p = pmlpp.tile([H, MLP_CH], f32, tag="ps2", name="h2p")
                nc.tensor.matmul(out=h2p[:, :csz], lhsT=w1_sb[:],
                                 rhs=h1[:, :csz], start=True, stop=True)
                h2 = mpool.tile([H, MLP_CH], bf, tag="h2", name="h2")
                nc.scalar.activation(out=h2[:, :csz], in_=h2p[:, :csz],
                                     func=AF.Relu, bias=b1_sb[:])
                h3p = pmlpp.tile([H, MLP_CH], f32, tag="ps3", name="h3p")
                nc.tensor.matmul(out=h3p[:, :csz], lhsT=w2_sb[:],
                                 rhs=h2[:, :csz], start=True, stop=True)
                nc.scalar.activation(out=out_sb[:, c0:c0 + csz],
                                     in_=h3p[:, :csz],
                                     func=AF.Identity, bias=b2_sb[:])
                nc.sync.dma_start(out=out_d[:, c0:c0 + csz],
                                  in_=out_sb[:, c0:c0 + csz])
    return nc


# --------------------------------------------------------------------------
# Entry point
# --------------------------------------------------------------------------

def _run(inputs, trace=False, trace_kwargs=None):
    T, in_maps = pack_inputs(**inputs)
    nc = build_kernel(T)
    if not nc.is_finalized():
        nc.finalize()      # runs Bacc.compile(): reg alloc + wait legalization
    res = run_bass_kernel_spmd(
        nc, in_maps, core_ids=list(range(N_CORES)),
        trace=trace, **(trace_kwargs or {}),
    )
    outs = np.stack([np.asarray(res.results[c]["out"], np.float32)
                     for c in range(N_CORES)])          # [C, H, NPAD]
    out = outs.transpose(0, 2, 1)[:, :NPC].reshape(N_NODES, H)
    return np.ascontiguousarray(out, dtype=np.float32), res


def kernel(**inputs):
    out, _ = _run(inputs)
    return out


# revision 34
# speedup vs baseline: 1.1434x; 1.1434x over previous
"""Trainium2 Bass kernel for a GNN message-passing NodeModel.

reference semantics:
    agg = scatter_mean(e, edge_index[1], N)        # [N, h]
    x   = concat([agg, v], -1)                     # [N, 2h]
    out = (relu(relu(x@W0+b0)@W1+b1))@W2+b2        # [N, h]

Sharding: nodes are partitioned contiguously across the 8 cores
(6250 nodes/core).  Edges are routed (host side) to the core that owns
their *receiver*, so the segment-sum and the MLP are fully local per
core and no collective is needed.

Per core, receivers are grouped into W-node blocks (W=64).  Each
block's edges are packed into fixed-size tiles of 128 edges.  On
device, per block j:
  oh[p, t, n] = (rloc[p, t] == iota[n])            (ONE DVE tensor_tensor,
                                                    broadcast APs)
  ps[h, jW+n] += e_tile[p, h]^T @ oh[:, t, :]      (TensorE, f32 PSUM,
                                                    per 128-edge tile t)
Per 512-node chunk (8 blocks), the scatter-sum PSUM is turned into the
scatter-mean and written to SBUF in one DVE op:
  agg[h, n] = ps[h, n] * crecip[h, n]              (crecip = 1/max(deg,1)
                                                    replicated rows, from host)
The MLP then runs feature-major over the same chunks:
  h1 = relu(W0a^T@agg + W0b^T@v^T + b0); h2 = relu(W1^T@h1 + b1);
  out = W2^T@h2 + b2.
"""

import functools

import numpy as np
import ml_dtypes

import concourse.bass as bass
import concourse.mybir as mybir
from concourse import bacc
from concourse.tile import TileContext
from concourse.bass_utils import run_bass_kernel_spmd

BF16 = ml_dtypes.bfloat16

N_NODES = 50000
N_EDGES = 800000
H = 64
N_CORES = 8
NPC = N_NODES // N_CORES          # 6250 nodes per core
W = 64                            # node-block (one-hot window) size
NBLK = -(-NPC // W)               # 98 blocks per core
NPAD = NBLK * W                   # 6272 padded nodes per core
MLP_CH = 512                      # MLP chunk (nodes per chunk)
BLK_GRP = 2                       # node blocks fetched per edge DMA


# --------------------------------------------------------------------------
# Host-side packing
# --------------------------------------------------------------------------

def pack_inputs(v, edge_index, e, W0, b0, W1, b1, W2, b2,
                n_cores=N_CORES, npc=NPC, nblk=NBLK, npad=NPAD):
    """Shard + lay out the full inputs for the 8 cores.

    Returns (T, in_maps): T is the number of 128-edge tiles per node
    block, in_maps the list of per-core input dicts.
    """
    h = W0.shape[1]
    n_nodes = n_cores * npc
    n_edges = e.shape[0]

    recv = np.asarray(edge_index[1], dtype=np.int64)
    order = np.argsort(recv, kind="stable")
    recv_s = recv[order]

    counts = np.bincount(recv, minlength=n_nodes)
    crecip_node = (1.0 / np.maximum(counts, 1)).astype(np.float32)

    core = recv_s // npc
    loc = recv_s - core * npc
    blk = loc // W
    rloc = loc - blk * W                         # in [0, W)
    key = core * nblk + blk
    cnt_blk = np.bincount(key, minlength=n_cores * nblk)
    T = max(1, -(-int(cnt_blk.max()) // 128))

    starts = np.zeros(n_cores * nblk, np.int64)
    np.cumsum(cnt_blk[:-1], out=starts[1:])
    pos = np.arange(n_edges) - starts[key]
    p = pos % 128
    t = pos // 128

    arr = np.zeros((n_cores, nblk, 128, T, h), dtype=BF16)
    arr[core, blk, p, t] = np.asarray(e, np.float32)[order].astype(BF16)
    # group BLK_GRP adjacent blocks into one DMA record:
    # [nblk/G, 128, G*T*h] with each partition's line contiguous
    g = BLK_GRP
    arr = arr.reshape(n_cores, nblk // g, g, 128, T * h)
    arr = np.ascontiguousarray(arr.transpose(0, 1, 3, 2, 4))
    arr = arr.reshape(n_cores, nblk // g, 128, g * T * h)

    # per-edge local receiver index, bf16, [core, p, blk*T]
    meta = np.zeros((n_cores, 128, nblk, T), dtype=BF16)
    meta[core, p, blk, t] = rloc.astype(np.float32).astype(BF16)
    meta = meta.reshape(n_cores, 128, nblk * T)

    # int16 scatter indices t*W + rloc for the gpsimd local_scatter path,
    # padded to even T2 with -1 ("negative indices are ignored")
    T2 = T + (T % 2)
    sidx = np.full((n_cores, 128, nblk, T2), -1, dtype=np.int16)
    sidx[core, p, blk, t] = (t * W + rloc).astype(np.int16)
    sidx = sidx.reshape(n_cores, 128, nblk * T2)

    # 1/deg replicated down feature rows, [core, h, npad] bf16
    crp = np.zeros((n_cores, npad), np.float32)
    crp[:, :npc] = crecip_node.reshape(n_cores, npc)
    crecip = np.ascontiguousarray(
        np.broadcast_to(crp[:, None, :], (n_cores, h, npad))).astype(BF16)

    vp = np.zeros((n_cores, npad, h), np.float32)
    vp[:, :npc] = np.asarray(v, np.float32).reshape(n_cores, npc, h)
    vt = np.ascontiguousarray(vp.transpose(0, 2, 1)).astype(BF16)  # [C, h, npad]

    W0 = np.asarray(W0, np.float32)
    w0a = np.ascontiguousarray(W0[:h]).astype(BF16)
    w0b = np.ascontiguousarray(W0[h:]).astype(BF16)
    w1 = np.asarray(W1, np.float32).astype(BF16)
    w2 = np.asarray(W2, np.float32).astype(BF16)
    b0 = np.asarray(b0, np.float32).reshape(h, 1).copy()
    b1 = np.asarray(b1, np.float32).reshape(h, 1).copy()
    b2 = np.asarray(b2, np.float32).reshape(h, 1).copy()

    in_maps = []
    for c in range(n_cores):
        in_maps.append({
            "arr": arr[c], "meta": meta[c], "sidx": sidx[c],
            "vt": vt[c], "crecip": crecip[c],
            "w0a": w0a, "w0b": w0b, "w1": w1, "w2": w2,
            "b0": b0, "b1": b1, "b2": b2,
        })
    return T, in_maps


# --------------------------------------------------------------------------
# Device kernel
# --------------------------------------------------------------------------

@functools.lru_cache(maxsize=4)
def build_kernel(T, nblk=NBLK, npad=NPAD):
    bf = mybir.dt.bfloat16
    f32 = mybir.dt.float32
    AF = mybir.ActivationFunctionType

    nc = bacc.Bacc(trn_type="TRN2", debug=False)

    g = BLK_GRP
    arr_d = nc.declare_dram_parameter("arr", [nblk // g, 128, g * T * H], bf,
                                      isOutput=False)
    meta_d = nc.declare_dram_parameter("meta", [128, nblk * T], bf, isOutput=False)
    T2 = T + (T % 2)
    sidx_d = nc.declare_dram_parameter("sidx", [128, nblk * T2], mybir.dt.int16,
                                       isOutput=False)
    vt_d = nc.declare_dram_parameter("vt", [H, npad], bf, isOutput=False)
    cr_d = nc.declare_dram_parameter("crecip", [H, npad], bf, isOutput=False)
    w0a_d = nc.declare_dram_parameter("w0a", [H, H], bf, isOutput=False)
    w0b_d = nc.declare_dram_parameter("w0b", [H, H], bf, isOutput=False)
    w1_d = nc.declare_dram_parameter("w1", [H, H], bf, isOutput=False)
    w2_d = nc.declare_dram_parameter("w2", [H, H], bf, isOutput=False)
    b0_d = nc.declare_dram_parameter("b0", [H, 1], f32, isOutput=False)
    b1_d = nc.declare_dram_parameter("b1", [H, 1], f32, isOutput=False)
    b2_d = nc.declare_dram_parameter("b2", [H, 1], f32, isOutput=False)
    out_d = nc.declare_dram_parameter("out", [H, npad], f32, isOutput=True)

    n_ch = -(-npad // MLP_CH)
    blocks_per_ch = MLP_CH // W          # 8

    with TileContext(nc) as tc:
        with (
            tc.tile_pool(name="const", bufs=1) as cpool,
            tc.tile_pool(name="blocks", bufs=4) as bpool,
            tc.tile_pool(name="oh", bufs=10) as ohpool,
            tc.tile_pool(name="mlp", bufs=3) as mpool,
            tc.tile_pool(name="pseg", bufs=2, space="PSUM") as psegp,
            tc.tile_pool(name="pmlp", bufs=2, space="PSUM") as pmlpp,
        ):
            # iota row [0..W-1] per partition, built on-device
            iota_i16 = cpool.tile([128, W], mybir.dt.int16)
            nc.gpsimd.iota(out=iota_i16[:], pattern=[[1, W]], base=0,
                           channel_multiplier=0)
            iota_sb = cpool.tile([128, W], bf)
            nc.vector.tensor_copy(out=iota_sb[:], in_=iota_i16[:])
            meta_sb = cpool.tile([128, nblk * T], bf)
            nc.sync.dma_start(out=meta_sb[:], in_=meta_d[:])
            # absorb the meta DMA wait on DVE before the eq ops
            meta_probe = cpool.tile([128, 1], bf)
            nc.vector.tensor_copy(out=meta_probe[:], in_=meta_sb[:, 0:1])
            sidx_sb = cpool.tile([128, nblk * T2], mybir.dt.int16)
            nc.sync.dma_start(out=sidx_sb[:], in_=sidx_d[:])
            ones_sb = cpool.tile([128, T2], bf)
            nc.gpsimd.memset(ones_sb[:], 1.0)
            from concourse import library_config
            nc.gpsimd.load_library(library_config.local_scatter)

            vt_sb = cpool.tile([H, npad], bf)
            nc.scalar.dma_start(out=vt_sb[:], in_=vt_d[:])
            cr_sb = cpool.tile([H, npad], bf)
            nc.scalar.dma_start(out=cr_sb[:], in_=cr_d[:])
            w0a_sb = cpool.tile([H, H], bf)
            nc.scalar.dma_start(out=w0a_sb[:], in_=w0a_d[:])
            w0b_sb = cpool.tile([H, H], bf)
            nc.scalar.dma_start(out=w0b_sb[:], in_=w0b_d[:])
            w1_sb = cpool.tile([H, H], bf)
            nc.scalar.dma_start(out=w1_sb[:], in_=w1_d[:])
            w2_sb = cpool.tile([H, H], bf)
            nc.scalar.dma_start(out=w2_sb[:], in_=w2_d[:])
            b0_sb = cpool.tile([H, 1], f32)
            nc.scalar.dma_start(out=b0_sb[:], in_=b0_d[:])
            b1_sb = cpool.tile([H, 1], f32)
            nc.scalar.dma_start(out=b1_sb[:], in_=b1_d[:])
            b2_sb = cpool.tile([H, 1], f32)
            nc.scalar.dma_start(out=b2_sb[:], in_=b2_d[:])

            agg_tiles = [cpool.tile([H, MLP_CH], bf, tag=f"agg{i}",
                                    name=f"agg{i}")
                         for i in range(n_ch)]
            out_sb = cpool.tile([H, npad], f32)

            # ---- scatter-sum (per chunk of 8 blocks) + mean + MLP ----
            blk_tiles = {}

            def emit_oh(j):
                oh = ohpool.tile([128, T * W], bf, tag="oh", name="oh")
                if j < 2 * blocks_per_ch or j % 2 == 0:
                    rloc_ap = meta_sb[:, j * T:(j + 1) * T]  # [128, T]
                    nc.vector.tensor_tensor(
                        out=oh[:].rearrange("p (t w) -> p t w", w=W),
                        in0=rloc_ap.to_broadcast([128, T, W]),
                        in1=iota_sb[:].rearrange("p (o w) -> p o w", o=1)
                            .to_broadcast([128, T, W]),
                        op=mybir.AluOpType.is_equal,
                    )
                else:
                    nc.gpsimd.local_scatter(
                        out_ap=oh[:],
                        data_ap=ones_sb[:],
                        idxs_ap=sidx_sb[:, j * T2:(j + 1) * T2],
                        channels=128,
                        num_elems=T * W,
                        num_idxs=T2,
                    )
                return oh

            for ci in range(n_ch):
                ps = psegp.tile([H, MLP_CH], f32, tag="ps", name="ps")
                for bj in range(blocks_per_ch):
                    j = ci * blocks_per_ch + bj        # global block
                    if j >= nblk:
                        break
                    if j % g == 0:
                        blkt = bpool.tile([128, g * T * H], bf, tag="blk",
                                          name="blkt")
                        nc.sync.dma_start(out=blkt[:], in_=arr_d[j // g])
                        for i in range(g):
                            blk_tiles[j + i] = (blkt, i)
                    blkt, sub = blk_tiles[j]
                    base = sub * T * H
                    oh = emit_oh(j)
                    for t in range(T):
                        nc.tensor.matmul(
                            out=ps[:, bj * W:(bj + 1) * W],
                            lhsT=blkt[:, base + t * H: base + (t + 1) * H],
                            rhs=oh[:, t * W:(t + 1) * W],
                            start=(t == 0),
                            stop=(t == T - 1),
                        )
                c0 = ci * MLP_CH
                csz = min(MLP_CH, npad - c0)
                nc.vector.tensor_tensor(
                    out=agg_tiles[ci][:, :csz],
                    in0=ps[:, :csz],
                    in1=cr_sb[:, c0:c0 + csz],
                    op=mybir.AluOpType.mult,
                )
                h1p = pmlpp.tile([H, MLP_CH], f32, tag="ps1", name="h1p")
                nc.tensor.matmul(out=h1p[:, :csz], lhsT=w0a_sb[:],
                                 rhs=agg_tiles[ci][:, :csz],
                                 start=True, stop=False)
                nc.tensor.matmul(out=h1p[:, :csz], lhsT=w0b_sb[:],
                                 rhs=vt_sb[:, c0:c0 + csz],
                                 start=False, stop=True)
                h1 = mpool.tile([H, MLP_CH], bf, tag="h1", name="h1")
                nc.scalar.activation(out=h1[:, :csz], in_=h1p[:, :csz],
                                     func=AF.Relu, bias=b0_sb[:])
                h2p = pmlpp.tile([H, MLP_CH], f32, tag="ps2", name="h2p")
                nc.tensor.matmul(out=h2p[:, :csz], lhsT=w1_sb[:],
                                 rhs=h1[:, :csz], start=True, stop=True)
                h2 = mpool.tile([H, MLP_CH], bf, tag="h2", name="h2")
                nc.scalar.activation(out=h2[:, :csz], in_=h2p[:, :csz],
                                     func=AF.Relu, bias=b1_sb[:])
                h3p = pmlpp.tile([H, MLP_CH], f32, tag="ps3", name="h3p")
                nc.tensor.matmul(out=h3p[:, :csz], lhsT=w2_sb[:],
                                 rhs=h2[:, :csz], start=True, stop=True)
                nc.scalar.activation(out=out_sb[:, c0:c0 + csz],
                                     in_=h3p[:, :csz],
                                     func=AF.Identity, bias=b2_sb[:])
                # output flows out per 2 chunks on the ACT HWDGE queue so
                # it never delays edge-block fetches on the sync queue
                if ci % 2 == 1 or ci == n_ch - 1:
                    o0 = (ci - (ci % 2)) * MLP_CH if ci % 2 == 1 else c0
                    o0 = (ci // 2) * 2 * MLP_CH
                    nc.scalar.dma_start(out=out_d[:, o0:c0 + csz],
                                        in_=out_sb[:, o0:c0 + csz])
    return nc


# --------------------------------------------------------------------------
# Entry point
# --------------------------------------------------------------------------

def _run(inputs, trace=False, trace_kwargs=None):
    T, in_maps = pack_inputs(**inputs)
    nc = build_kernel(T)
    if not nc.is_finalized():
        nc.finalize()      # runs Bacc.compile(): reg alloc + wait legalization
    res = run_bass_kernel_spmd(
        nc, in_maps, core_ids=list(range(N_CORES)),
        trace=trace, **(trace_kwargs or {}),
    )
    outs = np.stack([np.asarray(res.results[c]["out"], np.float32)
                     for c in range(N_CORES)])          # [C, H, NPAD]
    out = outs.transpose(0, 2, 1)[:, :NPC].reshape(N_NODES, H)
    return np.ascontiguousarray(out, dtype=np.float32), res


def kernel(**inputs):
    out, _ = _run(inputs)
    return out
